# revision 31
# baseline (speedup 1.0000x reference)
"""Clustered attention kernel for Trainium2 (8 NeuronCores, SPMD).

Problem: nn_ClusteredAttention — softmax attention where query i may only
attend key j if label[i] == label[j] (8 labels), except the final "time"
token (index l-1) which attends everywhere and is attendable by everyone.

Strategy: block-diagonal attention over label clusters.
 - Host: per batch, group tokens by label into 8 clusters (~l/8 each).
   32 (batch, cluster) blocks are dealt to 8 cores x 4 slots (2 cores per
   batch; slots take the 2s-th / 2s+1-th largest cluster so one SPMD
   program with global per-slot capacities fits every core).
 - Packed per-core device inputs (fp16):
     qt [66, sum(pad_s)]  rows 0..63 = scale*q^T (cluster queries + time
                          query appended), row 64 = 1, row 65 = time-query
                          indicator.
     kt [66, sum(pad_s)]  rows 0..63 = k^T (cluster keys + time key
                          appended), row 64 = additive pad mask {0,-30},
                          row 65 = -30 at the time-key column (so the time
                          query does not double-count it across blocks).
     v  [128, sum(nch_s), 65]  values (col 64 = 1 -> softmax denominator
                          falls out of the AV matmul).
 - Device (per slot): scores^T = kt^T @ qt on PE (mask rides inside the
   matmul via the two extra contraction rows), exp on ScalarE
   (PSUM -> SBUF fp16), AV + denominator matmul on PE, reciprocal +
   per-partition normalize on VectorE, DMA out [128, qsubs, 65] fp32
   (col 64 = raw denominator).
 - Host: inverse-permute rows; the time query's row is emitted
   unnormalized per block (denominator column lets us undo the on-device
   divide), summed across its batch's 8 blocks, its self term added, then
   normalized.
"""

import math
import numpy as np

BIG = 30.0
_NCORES = 8
_NLABELS = 8

_prog_cache: dict[tuple, object] = {}


def _build_plan(label_arr, b, l):
    """Cluster index lists per (core, slot) + global per-slot capacities."""
    batch_clusters = []
    for bi in range(b):
        labels = np.asarray(label_arr[bi])
        cl = [np.nonzero(labels == c)[0] for c in range(_NLABELS)]
        cl.sort(key=lambda a: -len(a))
        batch_clusters.append(cl)
    core_slots = []  # [core][slot] = (bi, idx array)
    for bi in range(b):
        for half in range(2):
            core_slots.append(
                [(bi, batch_clusters[bi][2 * s + half]) for s in range(4)]
            )
    caps = tuple(
        max(len(core_slots[co][s][1]) + 1 for co in range(len(core_slots)))
        for s in range(4)
    )
    return caps, core_slots


def _pack_core(query, key, value, slots, caps, scale):
    """Build the three packed fp16 device arrays for one core."""
    l = query.shape[1]
    T = l - 1
    qts, kts, vs = [], [], []
    for (bi, idx), cap in zip(slots, caps):
        n = len(idx)
        pad = -(-cap // 128) * 128
        qt = np.zeros((66, pad), np.float32)
        qt[0:64, 0:n] = (query[bi, idx, :] * scale).T
        qt[0:64, n] = query[bi, T, :] * scale
        qt[64, :] = 1.0
        qt[65, n] = 1.0
        kt = np.zeros((66, pad), np.float32)
        kt[0:64, 0:n] = key[bi, idx, :].T
        kt[0:64, n] = key[bi, T, :]
        kt[64, n + 1:] = -BIG
        kt[65, n] = -BIG
        v = np.zeros((pad, 65), np.float32)
        v[0:n, 0:64] = value[bi, idx, :]
        v[n, 0:64] = value[bi, T, :]
        v[:, 64] = 1.0
        qts.append(qt.astype(np.float16))
        kts.append(kt.astype(np.float16))
        vs.append(v.reshape(pad // 128, 128, 65).transpose(1, 0, 2).astype(np.float16))
    return {
        "kq": np.stack(
            [np.concatenate(kts, axis=1), np.concatenate(qts, axis=1)], axis=1
        ),
        "v": np.concatenate(vs, axis=1),
    }


def _split_waits(nc, limit=1):
    """This container's walrus rejects >1 sync wait per instruction; move
    excess waits onto same-engine EventSemaphore carriers placed just
    before (per-engine program order is preserved, so semantics are too)."""
    import concourse.mybir as mybir

    n = 0
    for bl in nc.m.functions[0].blocks:
        insts = list(bl.instructions)
        new = []
        for i in insts:
            si = i.sync_info
            waits = list(si.on_wait) if (si is not None and si.on_wait) else []
            if len(waits) > limit:
                rest, keep = waits[:-limit], waits[-limit:]
                while rest:
                    grp, rest = rest[:limit], rest[limit:]
                    c = mybir.InstEventSemaphore(
                        name=f"waitcar_{n}", ins=[], outs=[]
                    )
                    n += 1
                    c.engine = i.engine
                    c.sync_info = mybir.SyncInfo(on_wait=grp, on_update=[])
                    new.append(c)
                i.sync_info = mybir.SyncInfo(
                    on_wait=keep, on_update=list(si.on_update or [])
                )
            new.append(i)
        bl.instructions = new
    return n


def _build_program(caps, fixup=True, repeat=1):
    import concourse.bass as bass
    import concourse.mybir as mybir
    import concourse.tile as tile

    f16 = mybir.dt.float16
    f32 = mybir.dt.float32
    pads = [-(-c // 128) * 128 for c in caps]
    TOT = sum(pads)
    NCH = sum(p // 128 for p in pads)

    nc = bass.Bass()
    kq_d = nc.declare_dram_parameter("kq", [66, 2, TOT], f16, isOutput=False)
    v_d = nc.declare_dram_parameter("v", [128, NCH, 65], f16, isOutput=False)
    out_d = nc.declare_dram_parameter("out", [128, NCH, 65], f32, isOutput=True)

    QCH = 256
    with tile.TileContext(nc) as tc:
        with (
            tc.tile_pool(name="inp", bufs=1) as inp,
            tc.tile_pool(name="epool", bufs=2) as epool,
            tc.tile_pool(name="opool", bufs=2) as opool,
            tc.tile_pool(name="small", bufs=4) as small,
            tc.tile_pool(name="psS", bufs=2, space="PSUM") as psS,
            tc.tile_pool(name="psO", bufs=2, space="PSUM") as psO,
        ):
          for rep in range(repeat):
            # dummy 1-element exp at t=0: hoists the ~1.4us ACT_TABLE_LOAD
            # off the critical path (it overlaps the input-DMA head)
            warm_in = small.tile([128, 1], f32, tag="warm", name=f"warm_in{rep}")
            nc.vector.memset(warm_in, 0.0)
            warm_out = small.tile([128, 1], f16, tag="warmo", name=f"warm_out{rep}")
            nc.scalar.activation(
                warm_out, warm_in, mybir.ActivationFunctionType.Exp
            )

            # one kt+qt load per slot on the SP ring (slot0 first so compute
            # starts after one small transfer, later slots stream during
            # compute); v on the otherwise-idle GPSIMD ring; the ACT ring
            # stays clear for exp
            kq_all = inp.tile([66, 2, TOT], f16, tag="kq", name=f"kq_all{rep}")
            v_all = inp.tile([128, NCH, 65], f16, tag="v", name=f"v_all{rep}")
            o = 0
            for s0 in range(len(caps)):
                p = pads[s0]
                if s0 == 0:
                    # split slot0's load across both HWDGE rings (SP + ACT):
                    # dispatch ends ~2x sooner, so the fixed completion
                    # receipt - which gates the first score matmul - starts
                    # earlier; the brief ACT-ring dispatch fits in the
                    # table-load slack
                    h = p // 2
                    nc.sync.dma_start(kq_all[:, :, 0:h], kq_d[:, :, 0:h])
                    nc.scalar.dma_start(kq_all[:, :, h:p], kq_d[:, :, h:p])
                else:
                    nc.sync.dma_start(kq_all[:, :, o:o + p], kq_d[:, :, o:o + p])
                o += p
            nc.gpsimd.dma_start(v_all, v_d[:])

            def emit_scores(s, s0, off):
                cap = caps[s0]
                pad = pads[s0]
                nch = pad // 128
                kt_t = kq_all[:, 0, off:off + pad]
                qt_t = kq_all[:, 1, off:off + pad]
                et = epool.tile([128, nch, pad], f16, tag="et", name=f"et{s}")
                # equal-width chunks (each has its own PSUM tile, so any
                # width <= QCH is bank-legal): every scores phase then fits
                # inside an exp window and the ACT pipe never stalls; the
                # very first chunk of slot 0 is narrow for fast ACT rampup
                nck = -(-cap // QCH)
                if s0 == 0 and rep == 0:
                    base, extra = divmod(cap - 64, nck)
                    widths = [64] + [
                        base + (1 if i < extra else 0) for i in range(nck)
                    ]
                else:
                    base, extra = divmod(cap, nck)
                    widths = [base + (1 if i < extra else 0) for i in range(nck)]
                cuts = [0]
                for w in widths:
                    cuts.append(cuts[-1] + w)
                for qci in range(len(cuts) - 1):
                    qc, qe = cuts[qci], cuts[qci + 1]
                    qw = qe - qc
                    ps = psS.tile([128, nch, QCH], f32, tag="ps", name=f"ps{s}_{qc}")
                    for kc in range(nch):
                        nc.tensor.matmul(
                            ps[:, kc, :qw],
                            lhsT=kt_t[:, kc * 128:(kc + 1) * 128],
                            rhs=qt_t[:, qc:qc + qw],
                            start=True,
                            stop=True,
                        )
                    nc.scalar.activation(
                        et[:, :, qc:qe],
                        ps[:, :, :qw],
                        mybir.ActivationFunctionType.Exp,
                    )
                if cap < pad:
                    # pad query columns: fabricate weight 1 on chunk-0 keys so
                    # the denominator is nonzero (rows are discarded by host)
                    nc.vector.memset(et[:, 0, cap:pad], 1.0)
                    if nch > 1:
                        nc.vector.memset(et[:, 1:, cap:pad], 0.0)
                return et

            def av_one(s, s0, qs, et, v_t, ob):
                nch = pads[s0] // 128
                po = psO.tile([128, 65], f32, tag="po", name=f"po{s}_{qs}")
                for kc in range(nch):
                    nc.tensor.matmul(
                        po,
                        lhsT=et[:, kc, qs * 128:(qs + 1) * 128],
                        rhs=v_t[:, kc, :],
                        start=(kc == 0),
                        stop=(kc == nch - 1),
                    )
                rec = small.tile([128, 1], f32, tag="rec", name=f"rec{s}_{qs}")
                nc.vector.reciprocal(rec, po[:, 64:65])
                nc.vector.tensor_scalar_mul(ob[:, qs, 0:64], po[:, 0:64], rec)
                nc.vector.tensor_copy(ob[:, qs, 64:65], po[:, 64:65])

            def emit_av(s, s0, choff, et):
                nch = pads[s0] // 128
                v_t = v_all[:, choff:choff + nch, :]
                ob = opool.tile([128, nch, 65], f32, tag="ob", name=f"ob{s}")
                for qs in range(nch):
                    av_one(s, s0, qs, et, v_t, ob)
                nc.sync.dma_start(out_d[:, choff:choff + nch, :], ob)

            def emit_last(s, s0, off, choff, prev_args):
                # final slot: AV per-qsub as soon as the covering exp chunk
                # lands, so the kernel tail is one exp chunk + one AV, not a
                # whole slot's AV
                cap = caps[s0]
                pad = pads[s0]
                nch = pad // 128
                kt_t = kq_all[:, 0, off:off + pad]
                qt_t = kq_all[:, 1, off:off + pad]
                v_t = v_all[:, choff:choff + nch, :]
                et = epool.tile([128, nch, pad], f16, tag="et", name=f"et{s}")
                ob = opool.tile([128, nch, 65], f32, tag="ob", name=f"ob{s}")
                # equal-width chunks up to the last qsub boundary, then one
                # final chunk, so only the last qsub's AV waits on the last exp
                sp = min((nch - 1) * 128, cap)
                nck = max(1, -(-sp // QCH))
                base, extra = divmod(sp, nck)
                cuts = [0]
                for i in range(nck):
                    cuts.append(cuts[-1] + base + (1 if i < extra else 0))
                if cap > sp:
                    cuts.append(cap)
                nqs = 0
                flushed = False
                for qci in range(len(cuts) - 1):
                    qc, qe = cuts[qci], cuts[qci + 1]
                    qw = qe - qc
                    ps = psS.tile([128, nch, QCH], f32, tag="ps", name=f"ps{s}_{qc}")
                    for kc in range(nch):
                        nc.tensor.matmul(
                            ps[:, kc, :qw],
                            lhsT=kt_t[:, kc * 128:(kc + 1) * 128],
                            rhs=qt_t[:, qc:qc + qw],
                            start=True,
                            stop=True,
                        )
                    nc.scalar.activation(
                        et[:, :, qc:qe],
                        ps[:, :, :qw],
                        mybir.ActivationFunctionType.Exp,
                    )
                    if qci == 0 and prev_args is not None:
                        emit_av(*prev_args)
                    avail = min(qc // 128, nch)
                    for qs in range(nqs, avail):
                        av_one(s, s0, qs, et, v_t, ob)
                        if qs == nch - 2 and not flushed:
                            nc.sync.dma_start(
                                out_d[:, choff:choff + nch - 1, :],
                                ob[:, :nch - 1, :],
                            )
                            flushed = True
                    nqs = max(nqs, avail)
                if cap < pad:
                    nc.vector.memset(et[:, 0, cap:pad], 1.0)
                    if nch > 1:
                        nc.vector.memset(et[:, 1:, cap:pad], 0.0)
                for qs in range(nqs, nch):
                    av_one(s, s0, qs, et, v_t, ob)
                    if qs == nch - 2 and not flushed:
                        nc.sync.dma_start(
                            out_d[:, choff:choff + nch - 1, :],
                            ob[:, :nch - 1, :],
                        )
                        flushed = True
                nc.sync.dma_start(
                    out_d[:, choff + nch - 1:choff + nch, :],
                    ob[:, nch - 1:nch, :],
                )

            # software pipeline: AV of slot s-1 is emitted after scores/exp of
            # slot s, so next-slot score matmuls outrank AV in PE priority and
            # ACT never waits on an empty PSUM pipe
            prev = None
            off = 0
            choff = 0
            nslots = len(caps)
            for s0, cap in enumerate(caps):
                s = f"{rep}_{s0}"
                if s0 == nslots - 1:
                    emit_last(s, s0, off, choff, prev)
                    prev = None
                else:
                    et = emit_scores(s, s0, off)
                    if prev is not None:
                        emit_av(*prev)
                    prev = (s, s0, choff, et)
                off += pads[s0]
                choff += pads[s0] // 128
            if prev is not None:
                emit_av(*prev)
    if fixup:
        _split_waits(nc)
    return nc


def kernel(query, key, value, label_arr):
    query = np.ascontiguousarray(np.asarray(query, dtype=np.float32))
    key = np.ascontiguousarray(np.asarray(key, dtype=np.float32))
    value = np.ascontiguousarray(np.asarray(value, dtype=np.float32))
    label_np = np.asarray(label_arr)
    b, l, d = query.shape
    T = l - 1
    scale = 1.0 / math.sqrt(d)

    caps, core_slots = _build_plan(label_np, b, l)
    if caps not in _prog_cache:
        _prog_cache[caps] = _build_program(caps)
    nc = _prog_cache[caps]

    in_maps = [
        _pack_core(query, key, value, core_slots[co], caps, scale)
        for co in range(2 * b)
    ]

    from concourse.bass_utils import run_bass_kernel_spmd

    res = run_bass_kernel_spmd(nc, in_maps, core_ids=list(range(len(in_maps))))

    pads = [-(-c // 128) * 128 for c in caps]
    out = np.zeros((b, l, d), np.float32)
    U_T = np.zeros((b, d), np.float64)
    D_T = np.zeros((b,), np.float64)
    for co in range(2 * b):
        arr = res.results[co]["out"]  # [128, NCH, 65]
        choff = 0
        for s, (bi, idx) in enumerate(core_slots[co]):
            nch = pads[s] // 128
            blk = arr[:, choff:choff + nch, :].transpose(1, 0, 2).reshape(-1, 65)
            n = len(idx)
            out[bi, idx, :] = blk[0:n, 0:64]
            U_T[bi] += blk[n, 0:64].astype(np.float64) * blk[n, 64]
            D_T[bi] += blk[n, 64]
            choff += nch
    for bi in range(b):
        e_tt = math.exp(scale * float(np.dot(query[bi, T], key[bi, T])))
        U_T[bi] += e_tt * value[bi, T].astype(np.float64)
        D_T[bi] += e_tt
        out[bi, T] = (U_T[bi] / D_T[bi]).astype(np.float32)
    return out
